# revision 26
# baseline (speedup 1.0000x reference)
"""Trainium2 Bass kernel for nn_CTCPerSpeakerExtractorConcatNNG (v2).

Sharding: 8 cores = (batch b, T-half th). Each core computes the shared
X/KV/K/V once for its T-half (+halo) and both speaker streams' attention
+ FFN for its 768 query rows. No collectives; host scatters/gathers.

Geometry: xmT input is zero-padded by 128 rows on both ends of T, and a
core's slice is padded cols [th*768, th*768+1024). Query tiles are local
tiles 1..6; every tile's key window is ws=128*lt-24 (uniform), boundary
masking is data (host-built 0/1 band masks that also clamp to [0,T)).

Dataflow per core ([T-tiles x 128 part, D free] natural, bf16 acts):
  A) X = xmT.T @ Win (+bin); LN_kv -> lnkv; Xk[k] = X * What'[k] (x128)
  B) KVT transpose; C) KT = Wk^ @ KVT (+bk), Vh halo tiles (V @ ws grid)
  per stream k: lnq -> LNQT -> QT (+bq); banded attention with
    multiplicative masks (exp, mask*accum den on DVE, 1/den on gpsimd);
    y2 = Xk + YT.T@Wo' (Wo' = 128*Wo); LN_f -> LNFT (fp8)
    FFN in fp8 DoubleRow: H1 = gelu(psum/128 + b1) (W1' = 128*W1),
    y3 = y2 + H1 @ W2' + 128*b2k; LN_s normalized only (host affine).
  LN rstd everywhere = exp(-0.5*ln(var+eps)) -- stays in the ln/exp
  activation-table set shared with softmax exp (no table thrash).
LN gains/biases for kv/q/f folded into the following matmuls on host.
"""
import sys

for _p in ("/opt/trn_rl_repo", "/root/.axon_site/_ro/trn_rl_repo"):
    if _p not in sys.path:
        sys.path.append(_p)

from contextlib import ExitStack

import numpy as np
import ml_dtypes

import concourse.bass as bass
import concourse.bacc as bacc
import concourse.tile as tile
from concourse import mybir
from concourse.bass_utils import run_bass_kernel_spmd
from concourse.masks import make_identity

BF = mybir.dt.bfloat16
F32 = mybir.dt.float32
FP8 = mybir.dt.float8e4
AF = mybir.ActivationFunctionType
OP = mybir.AluOpType
DR = mybir.MatmulPerfMode.DoubleRow

B, T, D, KSP, H, BAND = 4, 1536, 512, 2, 8, 24
DH = D // H          # 64
P = 128
WIN = P + 2 * BAND   # 176
NC_D = D // P        # 4 chunks of contraction dim
DFF = 4 * D          # 2048
EPS = 1e-5

NQT = 6              # query tiles per stream (local tiles 1..6)
NSH = 8              # shared tiles (local rows [0, 1024))
TSH = NSH * P        # 1024
TQ = NQT * P         # 768
NV = 7               # V halo tiles at starts 104 + 128*j
SC = 128.0           # fp8 weight scale (folded exactly; see module doc)


def _bcast_ap(dram_ap, parts=128):
    """[N] dram vector -> [parts, N] broadcast AP (partition step 0)."""
    return bass.AP(
        tensor=dram_ap.tensor,
        offset=dram_ap.offset,
        ap=[[0, parts]] + list(dram_ap.ap),
    )


def build_program(add_bo: bool, add_bin: bool = False,
                  add_bv: bool = False) -> bass.Bass:
    nc = bacc.Bacc()

    # ---- DRAM I/O ----
    xmT = nc.dram_tensor("xmT", [D, TSH], BF, kind="ExternalInput")
    Wd = {}
    for nm, (di, do) in [("Win", (D, D)), ("Wq", (D, D)), ("Wk", (D, D)),
                         ("Wv", (D, D)), ("Wo", (D, D))]:
        Wd[nm] = nc.dram_tensor(nm, [di, do], BF, kind="ExternalInput")
    W1d = nc.dram_tensor("W1", [D, DFF], BF, kind="ExternalInput")
    W2d = nc.dram_tensor("W2", [DFF, D], BF, kind="ExternalInput")
    smalls_d = nc.dram_tensor("smalls", [P, 36], F32, kind="ExternalInput")
    rows_d = nc.dram_tensor("rows", [5, D], F32, kind="ExternalInput")
    masks_d = nc.dram_tensor("masks", [P, NQT * WIN], BF, kind="ExternalInput")
    out_d = nc.dram_tensor("out", [2 * TQ, D], F32, kind="ExternalOutput")
    out_t = out_d.rearrange("(n p) d -> n p d", p=P)

    with tile.TileContext(nc) as tc, ExitStack() as ctx:
        consts = ctx.enter_context(tc.tile_pool(name="consts", bufs=1))
        wpool = ctx.enter_context(tc.tile_pool(name="wpool", bufs=1))
        ktp = ctx.enter_context(tc.tile_pool(name="ktp", bufs=1))
        acts = ctx.enter_context(tc.tile_pool(name="acts", bufs=1))
        stream_p = ctx.enter_context(tc.tile_pool(name="stream_p", bufs=1))
        ln_nat_p = ctx.enter_context(tc.tile_pool(name="ln_nat_p", bufs=2))
        tT_p = ctx.enter_context(tc.tile_pool(name="tT_p", bufs=1))
        h1p = ctx.enter_context(tc.tile_pool(name="h1p", bufs=1))
        small = ctx.enter_context(tc.tile_pool(name="small", bufs=6))
        sm2 = ctx.enter_context(tc.tile_pool(name="sm2", bufs=3))
        outp = ctx.enter_context(tc.tile_pool(name="outp", bufs=3))
        psA = ctx.enter_context(tc.tile_pool(name="psA", bufs=3, space="PSUM"))
        psB = ctx.enter_context(tc.tile_pool(name="psB", bufs=2, space="PSUM"))
        psC = ctx.enter_context(tc.tile_pool(name="psC", bufs=2, space="PSUM"))
        psD = ctx.enter_context(tc.tile_pool(name="psD", bufs=1, space="PSUM"))

        # ---- constants ----
        ident = consts.tile([P, P], BF)
        make_identity(nc, ident)
        eps_t = consts.tile([P, 1], F32, tag="eps_t")
        nc.vector.memset(eps_t, EPS)

        xmT_s = ln_nat_p.tile([P, NC_D, TSH], BF, tag="ln_nat")
        nc.sync.dma_start(out=xmT_s, in_=xmT.rearrange("(c p) t -> p c t", p=P))

        # ---- weights/biases to SBUF (order: earliest-needed first) ----
        Ws = {}
        for nm in ("Win", "Wk", "Wv", "Wq", "Wo"):
            t = wpool.tile([P, NC_D, D], BF, tag=nm)
            eng = nc.sync if nm == "Win" else nc.scalar
            eng.dma_start(out=t, in_=Wd[nm].rearrange("(c p) o -> p c o", p=P))
            Ws[nm] = t
        W1s = wpool.tile([P, NC_D, DFF], BF, tag="W1")
        nc.scalar.dma_start(out=W1s, in_=W1d.rearrange("(c p) o -> p c o", p=P))
        W2s = wpool.tile([P, 16, D], BF, tag="W2")
        nc.scalar.dma_start(out=W2s, in_=W2d.rearrange("(c p) o -> p c o", p=P))

        # packed smalls: cols [0:12]=What' (12 q-tile cols: k*6+lt-1),
        # [12:16]=bq4, [16:20]=bk4, [20:36]=b1_16
        smalls = consts.tile([P, 36], F32, tag="smalls")
        nc.sync.dma_start(out=smalls, in_=smalls_d[:, :])
        What = smalls[:, 0:12]
        bq4 = smalls[:, 12:16]
        bk4 = smalls[:, 16:20]
        b1_16 = smalls[:, 20:36]
        masks = consts.tile([P, NQT, WIN], BF, tag="masks")
        nc.sync.dma_start(out=masks, in_=masks_d.rearrange("p (n w) -> p n w", n=NQT))
        # bias rows broadcast
        bin_b = consts.tile([P, D], F32, tag="bin_b")
        nc.sync.dma_start(out=bin_b, in_=_bcast_ap(rows_d[0, :]))
        bv_b = consts.tile([P, D], F32, tag="bv_b")
        nc.sync.dma_start(out=bv_b, in_=_bcast_ap(rows_d[1, :]))
        b2k_b = []
        for k in range(2):
            t = consts.tile([P, D], F32, tag=f"b2k{k}_b")
            nc.sync.dma_start(out=t, in_=_bcast_ap(rows_d[2 + k, :]))
            b2k_b.append(t)
        if add_bo:
            ones_r = consts.tile([1, P], BF, tag="ones_r")
            nc.vector.memset(ones_r, 1.0)
            bo_rf = consts.tile([1, D], F32, tag="bo_rf")
            nc.sync.dma_start(out=bo_rf, in_=rows_d[4:5, :])
            bo_rb = consts.tile([1, D], BF, tag="bo_rb")
            nc.vector.tensor_copy(out=bo_rb, in_=bo_rf)

        def ln_stats(in_ap, mv_ap):
            """bn_stats+aggr into mv_ap [128,2] = [mean, var]."""
            st = small.tile([P, 6], F32, tag="st6")
            nc.vector.bn_stats(out=st, in_=in_ap)
            nc.vector.bn_aggr(out=mv_ap, in_=st)

        def ln_rstd(in_ap):
            """mean[128,1], rstd[128,1] (per-tile; Sqrt + reciprocal)."""
            mv = small.tile([P, 2], F32, tag="mv")
            ln_stats(in_ap, mv)
            sd = small.tile([P, 1], F32, tag="sd")
            nc.scalar.activation(out=sd, in_=mv[:, 1:2], func=AF.Sqrt, bias=eps_t)
            rstd = small.tile([P, 1], F32, tag="rstd")
            nc.vector.reciprocal(out=rstd, in_=sd)
            return mv[:, 0:1], rstd

        def batch_rsqrt(mvb, n):
            """mvb [128, 2n] pairs -> rstd [128, n] in one ACT Sqrt + recip."""
            vcols = mvb.rearrange("p (t two) -> p two t", two=2)[:, 1, 0:n]
            sd = small.tile([P, 8], F32, tag="sdb")
            nc.scalar.activation(out=sd[:, 0:n], in_=vcols, func=AF.Sqrt, bias=eps_t)
            rst = small.tile([P, 8], F32, tag="rstb")
            nc.vector.reciprocal(out=rst[:, 0:n], in_=sd[:, 0:n])
            return rst

        # ---- A) X, LN_kv, Xk (both streams) ----
        lnkv = ln_nat_p.tile([P, NSH, D], BF, tag="ln_nat")
        Xk = acts.tile([P, 12, D], BF, tag="Xk")
        for mt in range(NSH):
            ps = psA.tile([P, D], F32, tag="psA")
            for c in range(NC_D):
                nc.tensor.matmul(
                    ps, lhsT=xmT_s[:, c, mt * P:(mt + 1) * P], rhs=Ws["Win"][:, c, :],
                    start=(c == 0), stop=(c == NC_D - 1))
            if add_bin:
                psb = sm2.tile([P, D], F32, tag="Xpsb")
                nc.vector.tensor_tensor(out=psb, in0=ps, in1=bin_b, op=OP.add)
            else:
                psb = ps
            mean, rstd = ln_rstd(psb)
            nc.vector.tensor_scalar(
                out=lnkv[:, mt, :], in0=psb, scalar1=mean, scalar2=rstd,
                op0=OP.subtract, op1=OP.mult)
            if 1 <= mt <= NQT:
                nc.scalar.activation(
                    out=Xk[:, mt - 1, :], in_=psb, func=AF.Copy,
                    scale=What[:, mt - 1:mt])
                nc.scalar.activation(
                    out=Xk[:, NQT + mt - 1, :], in_=psb, func=AF.Copy,
                    scale=What[:, NQT + mt - 1:NQT + mt])

        # ---- B) transpose LN_kv -> KVT [128, 4, TSH] ----
        def transpose_nat_to_T(src, dst, n, out_dt=BF):
            for mt in range(n):
                nc.sync.dma_start_transpose(
                    out=dst[:, :, mt * P:(mt + 1) * P], in_=src[:, mt, :])

        KVT = tT_p.tile([P, NC_D, TSH], BF, tag="tT")
        transpose_nat_to_T(lnkv, KVT, NSH)

        # ---- C) KT [128, 4, TSH] and V halo tiles ----
        KT = ktp.tile([P, NC_D, TSH], BF, tag="KT")
        for co in range(NC_D):
            for tch in range(2):
                ps = psA.tile([P, D], F32, tag="psA")
                for c in range(NC_D):
                    nc.tensor.matmul(
                        ps, lhsT=Ws["Wk"][:, c, co * P:(co + 1) * P],
                        rhs=KVT[:, c, tch * D:(tch + 1) * D],
                        start=(c == 0), stop=(c == NC_D - 1))
                nc.scalar.activation(
                    out=KT[:, co, tch * D:(tch + 1) * D], in_=ps,
                    func=AF.Identity, bias=bk4[:, co:co + 1])

        Vh = acts.tile([P, NV, D], BF, tag="Vh")
        for j in range(NV):
            s = 104 + j * P
            ps = psA.tile([P, D], F32, tag="psA")
            for c in range(NC_D):
                nc.tensor.matmul(
                    ps, lhsT=KVT[:, c, s:s + P], rhs=Ws["Wv"][:, c, :],
                    start=(c == 0), stop=(c == NC_D - 1))
            if add_bv:
                nc.vector.tensor_tensor(out=Vh[:, j, :], in0=ps, in1=bv_b, op=OP.add)
            else:
                nc.scalar.copy(out=Vh[:, j, :], in_=ps)

        inv_sqrt_dh = 1.0 / float(np.sqrt(DH))

        # ---- D') QT = Wq_eff^T @ KVT (+bq) -- LN(Xk) == LN(X), shared by
        # both streams (positive per-row gate cancels in LayerNorm)
        QT = ktp.tile([P, NC_D, TQ], BF, tag="QT")
        for co in range(NC_D):
            for tch, (t0w, w) in enumerate(((0, D), (D, TQ - D))):
                ps = psA.tile([P, D], F32, tag="psA")
                for c in range(NC_D):
                    nc.tensor.matmul(
                        ps[:, 0:w], lhsT=Ws["Wq"][:, c, co * P:(co + 1) * P],
                        rhs=KVT[:, c, P + t0w:P + t0w + w],
                        start=(c == 0), stop=(c == NC_D - 1))
                nc.scalar.activation(
                    out=QT[:, co, t0w:t0w + w], in_=ps[:, 0:w],
                    func=AF.Identity, bias=bq4[:, co:co + 1])

        # ---- E) attention (shared across streams) ----
        YT = acts.tile([P, NC_D, TQ], BF, tag="YT")
        for lt in range(1, NQT + 1):
            ws = lt * P - BAND
            q0 = (lt - 1) * P
            den = small.tile([P, H], F32, tag="den")
            pm_a = sm2.tile([P, H, WIN], BF, tag="pm_a")
            for h in range(H):
                hp, hc = 64 * (h % 2), h // 2
                ps = psB.tile([P, WIN], F32, tag="psB")
                nc.tensor.matmul(
                    ps, lhsT=QT[hp:hp + 64, hc, q0:q0 + P],
                    rhs=KT[hp:hp + 64, hc, ws:ws + WIN], start=True, stop=False)
                # accumulate additive band mask: ps += ident.T @ mask = mask
                nc.tensor.matmul(ps, lhsT=ident, rhs=masks[:, lt - 1, :],
                                 start=False, stop=True)
                nc.scalar.activation(out=pm_a[:, h, :], in_=ps, func=AF.Exp,
                                     scale=inv_sqrt_dh,
                                     accum_out=den[:, h:h + 1])
            r8 = small.tile([P, H], F32, tag="r8")
            nc.vector.reciprocal(out=r8, in_=den)
            psy = psD.tile([P, D], F32, tag="psD")
            for h in range(H):
                hp, hc = 64 * (h % 2), h // 2
                pms = sm2.tile([P, WIN], BF, tag="pms")
                nc.vector.tensor_scalar_mul(
                    out=pms, in0=pm_a[:, h, :], scalar1=r8[:, h:h + 1])
                ptp = psC.tile([P, 2 * P], BF, tag="psC")
                nc.tensor.transpose(ptp[:, 0:P], pms[:, 0:P], ident)
                nc.tensor.transpose(ptp[0:48, P:2 * P], pms[:, P:WIN], ident)
                pts = sm2.tile([P, 2 * P], BF, tag="pts")
                nc.vector.tensor_copy(out=pts, in_=ptp)
                nc.tensor.matmul(
                    psy[hp:hp + 64, hc * P:(hc + 1) * P],
                    lhsT=Vh[:, lt - 1, h * DH:(h + 1) * DH], rhs=pts[:, 0:P],
                    start=True, stop=False)
                nc.tensor.matmul(
                    psy[hp:hp + 64, hc * P:(hc + 1) * P],
                    lhsT=Vh[0:48, lt, h * DH:(h + 1) * DH],
                    rhs=pts[0:48, P:2 * P],
                    start=False, stop=True)
            nc.vector.tensor_copy(
                out=YT[:, :, q0:q0 + P],
                in_=psy.rearrange("p (c q) -> p c q", c=NC_D))

        # ---- F0) yo = YT.T @ Wo' (+bo') once, shared ----
        yo = acts.tile([P, NQT, D], BF, tag="yo")
        for mt in range(NQT):
            ps = psA.tile([P, D], F32, tag="psA")
            for c in range(NC_D):
                nc.tensor.matmul(
                    ps, lhsT=YT[:, c, mt * P:(mt + 1) * P], rhs=Ws["Wo"][:, c, :],
                    start=(c == 0), stop=(c == NC_D - 1 and not add_bo))
            if add_bo:
                nc.tensor.matmul(ps, lhsT=ones_r[:, 0:P], rhs=bo_rb,
                                 start=False, stop=True)
            nc.scalar.copy(out=yo[:, mt, :], in_=ps)

        # ---- F) y2_k = Xk + yo ; LN_f -> LNFT (both streams together) ----
        LNFT = tT_p.tile([P, NC_D, 2 * TQ], BF, tag="lnfT")
        y2 = stream_p.tile([P, 2 * NQT, D], BF, tag="y2")
        for k in range(2):
            lnf = ln_nat_p.tile([P, NQT, D], BF, tag="ln_nat")
            mvf = small.tile([P, 12], F32, tag="mvb")
            for grp in range(2):
                for mt in range(grp * 3, grp * 3 + 3):
                    eng = nc.vector if mt % 2 == 0 else nc.gpsimd
                    eng.tensor_tensor(
                        out=y2[:, k * NQT + mt, :], in0=yo[:, mt, :],
                        in1=Xk[:, k * NQT + mt, :], op=OP.add)
                    ln_stats(y2[:, k * NQT + mt, :], mvf[:, 2 * mt:2 * mt + 2])
                rstf = batch_rsqrt(mvf[:, grp * 6:grp * 6 + 6], 3)
                for mt in range(grp * 3, grp * 3 + 3):
                    j = mt - grp * 3
                    nc.vector.tensor_scalar(
                        out=lnf[:, mt, :], in0=y2[:, k * NQT + mt, :],
                        scalar1=mvf[:, 2 * mt:2 * mt + 1], scalar2=rstf[:, j:j + 1],
                        op0=OP.subtract, op1=OP.mult)
            for mt in range(NQT):
                nc.scalar.dma_start_transpose(
                    out=LNFT[:, :, (k * NQT + mt) * P:(k * NQT + mt + 1) * P],
                    in_=lnf[:, mt, :])

        # ---- G) FFN over all 12 q-tiles (both streams) ----
        for tch in range(3):
            H1g = h1p.tile([P, 16, D], BF, tag="H1g")
            for dh in range(16):
                ps = psA.tile([P, D], F32, tag="psA")
                for c in range(NC_D):
                    nc.tensor.matmul(
                        ps, lhsT=W1s[:, c, dh * P:(dh + 1) * P],
                        rhs=LNFT[:, c, tch * D:(tch + 1) * D],
                        start=(c == 0), stop=(c == NC_D - 1))
                nc.scalar.activation(out=H1g[:, dh, :], in_=ps,
                                     func=AF.Gelu, scale=1.0 / SC,
                                     bias=b1_16[:, dh:dh + 1])
            for s0 in range(0, 4, 2):
                mvg = small.tile([P, 4], F32, tag="mvg")
                y3bs = []
                for j in range(2):
                    mtg = tch * 4 + s0 + j
                    ps = psA.tile([P, D], F32, tag="psA")
                    for dh in range(16):
                        nc.tensor.matmul(
                            ps, lhsT=H1g[:, dh, (s0 + j) * P:(s0 + j + 1) * P],
                            rhs=W2s[:, dh, :], start=(dh == 0), stop=(dh == 15))
                    y3 = outp.tile([P, D], F32, tag="y3")
                    nc.vector.tensor_tensor(out=y3, in0=ps, in1=y2[:, mtg, :],
                                            op=OP.add)
                    y3b = outp.tile([P, D], F32, tag="y3b")
                    nc.vector.tensor_tensor(out=y3b, in0=y3,
                                            in1=b2k_b[mtg // NQT], op=OP.add)
                    ln_stats(y3b, mvg[:, 2 * j:2 * j + 2])
                    y3bs.append(y3b)
                rstg = batch_rsqrt(mvg, 2)
                for j in range(2):
                    mtg = tch * 4 + s0 + j
                    o_sb = outp.tile([P, D], F32, tag="o_sb")
                    nc.vector.tensor_scalar(
                        out=o_sb, in0=y3bs[j], scalar1=mvg[:, 2 * j:2 * j + 1],
                        scalar2=rstg[:, j:j + 1],
                        op0=OP.subtract, op1=OP.mult)
                    nc.sync.dma_start(out=out_t[mtg], in_=o_sb)

    nc.finalize()
    return nc


_PROG_CACHE = {}


def kernel(**inputs) -> np.ndarray:
    f32 = np.float32
    bf = ml_dtypes.bfloat16
    fp8 = ml_dtypes.float8_e4m3
    x_m = np.asarray(inputs["x_m"], f32)
    A = np.asarray(inputs["A"], f32)
    g = {kk: np.asarray(v, f32) for kk, v in inputs.items()}

    # fold LN affine params into following matmuls (exact algebra)
    Wq = g["ln_q_g"][:, None] * g["Wq"]
    bq = g["bq"] + g["ln_q_b"] @ g["Wq"]
    Wk = g["ln_kv_g"][:, None] * g["Wk"]
    bk = g["bk"] + g["ln_kv_b"] @ g["Wk"]
    Wv = g["ln_kv_g"][:, None] * g["Wv"]
    bv = g["bv"] + g["ln_kv_b"] @ g["Wv"]
    W1 = g["ln_f_g"][:, None] * g["W1"]
    b1 = g["b1"] + g["ln_f_b"] @ g["W1"]

    add_bo = bool(np.any(g["bo"] != 0.0))
    add_bin = bool(np.any(g["b_in"] != 0.0))
    add_bv = bool(np.any(bv != 0.0))
    key = (add_bo, add_bin, add_bv)
    if key not in _PROG_CACHE:
        _PROG_CACHE[key] = build_program(add_bo, add_bin=add_bin, add_bv=add_bv)
    nc = _PROG_CACHE[key]

    common = {
        "Win": np.ascontiguousarray(g["W_in"].astype(bf)),
        "Wq": np.ascontiguousarray(Wq.astype(bf)),
        "Wk": np.ascontiguousarray(Wk.astype(bf)),
        "Wv": np.ascontiguousarray(Wv.astype(bf)),
        "Wo": np.ascontiguousarray((SC * g["Wo"]).astype(bf)),
        "W1": np.ascontiguousarray((SC * W1).astype(bf)),
        "W2": np.ascontiguousarray((SC * g["W2"]).astype(bf)),
    }

    # gate: What' = 128*sigmoid(6(A-0.5))  [B, T, K]
    What = SC / (1.0 + np.exp(-6.0 * (A - 0.5)))
    # padded transposed input [B, 512, T+256]
    xmp = np.zeros((B, D, T + 2 * P), f32)
    xmp[:, :, P:P + T] = np.transpose(x_m, (0, 2, 1))

    # band masks per (th, lt): [128 q-part, 176 key-window]
    jj = np.arange(WIN)
    pp = np.arange(P)
    band = ((jj[None, :] >= pp[:, None]) & (jj[None, :] <= pp[:, None] + 2 * BAND))

    in_maps = []
    for c in range(8):
        b, th = c // 2, c % 2
        im = dict(common)
        im["xmT"] = np.ascontiguousarray(
            xmp[b, :, th * TQ:th * TQ + TSH].astype(bf))
        sm = np.zeros((P, 36), f32)
        for k in range(2):
            for lt in range(NQT):
                sm[:, k * NQT + lt] = What[b, th * TQ + lt * P:th * TQ + (lt + 1) * P, k]
        sm[:, 12:16] = bq.reshape(4, P).T
        sm[:, 16:20] = bk.reshape(4, P).T
        sm[:, 20:36] = b1.reshape(16, P).T
        im["smalls"] = sm
        rows = np.stack([g["b_in"], bv,
                         SC * (g["b2"] + g["spk_tags"][0]),
                         SC * (g["b2"] + g["spk_tags"][1]),
                         SC * g["bo"]])
        im["rows"] = rows.astype(f32)
        mk = np.zeros((P, NQT, WIN), f32)
        for lt in range(1, NQT + 1):
            ws_true = th * TQ + lt * P - BAND - P  # true T coord of window col 0
            valid = (jj[None, :] + ws_true >= 0) & (jj[None, :] + ws_true < T)
            mk[:, lt - 1, :] = np.where(band & valid, 0.0, -1e30)
        im["masks"] = np.ascontiguousarray(mk.reshape(P, NQT * WIN).astype(bf))
        in_maps.append(im)

    res = run_bass_kernel_spmd(nc, in_maps, core_ids=list(range(8)))
    out = np.zeros((B, KSP * T, D), f32)
    gs, bs = g["ln_s_g"], g["ln_s_b"]
    for c in range(8):
        b, th = c // 2, c % 2
        r = res.results[c]["out"]
        for k in range(2):
            out[b, k * T + th * TQ:k * T + (th + 1) * TQ] = \
                r[k * TQ:(k + 1) * TQ] * gs + bs
    return out


# revision 27
# speedup vs baseline: 1.0463x; 1.0463x over previous
"""Trainium2 Bass kernel for nn_CTCPerSpeakerExtractorConcatNNG (v2).

Sharding: 8 cores = (batch b, T-half th). Each core computes the shared
X/KV/K/V once for its T-half (+halo) and both speaker streams' attention
+ FFN for its 768 query rows. No collectives; host scatters/gathers.

Geometry: xmT input is zero-padded by 128 rows on both ends of T, and a
core's slice is padded cols [th*768, th*768+1024). Query tiles are local
tiles 1..6; every tile's key window is ws=128*lt-24 (uniform), boundary
masking is data (host-built 0/1 band masks that also clamp to [0,T)).

Dataflow per core ([T-tiles x 128 part, D free] natural, bf16 acts):
  A) X = xmT.T @ Win (+bin); LN_kv -> lnkv; Xk[k] = X * What'[k] (x128)
  B) KVT transpose; C) KT = Wk^ @ KVT (+bk), Vh halo tiles (V @ ws grid)
  per stream k: lnq -> LNQT -> QT (+bq); banded attention with
    multiplicative masks (exp, mask*accum den on DVE, 1/den on gpsimd);
    y2 = Xk + YT.T@Wo' (Wo' = 128*Wo); LN_f -> LNFT (fp8)
    FFN in fp8 DoubleRow: H1 = gelu(psum/128 + b1) (W1' = 128*W1),
    y3 = y2 + H1 @ W2' + 128*b2k; LN_s normalized only (host affine).
  LN rstd everywhere = exp(-0.5*ln(var+eps)) -- stays in the ln/exp
  activation-table set shared with softmax exp (no table thrash).
LN gains/biases for kv/q/f folded into the following matmuls on host.
"""
import sys

for _p in ("/opt/trn_rl_repo", "/root/.axon_site/_ro/trn_rl_repo"):
    if _p not in sys.path:
        sys.path.append(_p)

from contextlib import ExitStack

import numpy as np
import ml_dtypes

import concourse.bass as bass
import concourse.bacc as bacc
import concourse.tile as tile
from concourse import mybir
from concourse.bass_utils import run_bass_kernel_spmd
from concourse.masks import make_identity

BF = mybir.dt.bfloat16
F32 = mybir.dt.float32
FP8 = mybir.dt.float8e4
AF = mybir.ActivationFunctionType
OP = mybir.AluOpType
DR = mybir.MatmulPerfMode.DoubleRow

B, T, D, KSP, H, BAND = 4, 1536, 512, 2, 8, 24
DH = D // H          # 64
P = 128
WIN = P + 2 * BAND   # 176
NC_D = D // P        # 4 chunks of contraction dim
DFF = 4 * D          # 2048
EPS = 1e-5

NQT = 6              # query tiles per stream (local tiles 1..6)
NSH = 8              # shared tiles (local rows [0, 1024))
TSH = NSH * P        # 1024
TQ = NQT * P         # 768
NV = 7               # V halo tiles at starts 104 + 128*j
SC = 128.0           # fp8 weight scale (folded exactly; see module doc)


def _bcast_ap(dram_ap, parts=128):
    """[N] dram vector -> [parts, N] broadcast AP (partition step 0)."""
    return bass.AP(
        tensor=dram_ap.tensor,
        offset=dram_ap.offset,
        ap=[[0, parts]] + list(dram_ap.ap),
    )


def build_program(add_bo: bool, add_bin: bool = False,
                  add_bv: bool = False) -> bass.Bass:
    nc = bacc.Bacc()

    # ---- DRAM I/O ----
    xmT = nc.dram_tensor("xmT", [D, TSH], BF, kind="ExternalInput")
    Wd = {}
    for nm, (di, do) in [("Win", (D, D)), ("Wq", (D, D)), ("Wk", (D, D)),
                         ("Wv", (D, D)), ("Wo", (D, D))]:
        Wd[nm] = nc.dram_tensor(nm, [di, do], BF, kind="ExternalInput")
    W1d = nc.dram_tensor("W1", [D, DFF], BF, kind="ExternalInput")
    W2d = nc.dram_tensor("W2", [DFF, D], BF, kind="ExternalInput")
    smalls_d = nc.dram_tensor("smalls", [P, 36], F32, kind="ExternalInput")
    rows_d = nc.dram_tensor("rows", [5, D], F32, kind="ExternalInput")
    masks_d = nc.dram_tensor("masks", [P, NQT * WIN], BF, kind="ExternalInput")
    out_d = nc.dram_tensor("out", [2 * TQ, D], F32, kind="ExternalOutput")
    out_t = out_d.rearrange("(n p) d -> n p d", p=P)

    with tile.TileContext(nc) as tc, ExitStack() as ctx:
        consts = ctx.enter_context(tc.tile_pool(name="consts", bufs=1))
        wpool = ctx.enter_context(tc.tile_pool(name="wpool", bufs=1))
        ktp = ctx.enter_context(tc.tile_pool(name="ktp", bufs=1))
        acts = ctx.enter_context(tc.tile_pool(name="acts", bufs=1))
        stream_p = ctx.enter_context(tc.tile_pool(name="stream_p", bufs=1))
        ln_nat_p = ctx.enter_context(tc.tile_pool(name="ln_nat_p", bufs=2))
        tT_p = ctx.enter_context(tc.tile_pool(name="tT_p", bufs=1))
        h1p = ctx.enter_context(tc.tile_pool(name="h1p", bufs=1))
        small = ctx.enter_context(tc.tile_pool(name="small", bufs=6))
        sm2 = ctx.enter_context(tc.tile_pool(name="sm2", bufs=3))
        outp = ctx.enter_context(tc.tile_pool(name="outp", bufs=3))
        psA = ctx.enter_context(tc.tile_pool(name="psA", bufs=3, space="PSUM"))
        psB = ctx.enter_context(tc.tile_pool(name="psB", bufs=2, space="PSUM"))
        psC = ctx.enter_context(tc.tile_pool(name="psC", bufs=2, space="PSUM"))
        psD = ctx.enter_context(tc.tile_pool(name="psD", bufs=1, space="PSUM"))

        # ---- constants ----
        ident = consts.tile([P, P], BF)
        make_identity(nc, ident)
        eps_t = consts.tile([P, 1], F32, tag="eps_t")
        nc.vector.memset(eps_t, EPS)

        xmT_s = ln_nat_p.tile([P, NC_D, TSH], BF, tag="ln_nat")
        nc.sync.dma_start(out=xmT_s, in_=xmT.rearrange("(c p) t -> p c t", p=P))

        # ---- weights/biases to SBUF (order: earliest-needed first) ----
        Ws = {}
        for nm in ("Win", "Wk", "Wv", "Wq", "Wo"):
            t = wpool.tile([P, NC_D, D], BF, tag=nm)
            eng = nc.sync if nm == "Win" else nc.scalar
            eng.dma_start(out=t, in_=Wd[nm].rearrange("(c p) o -> p c o", p=P))
            Ws[nm] = t
        W1s = wpool.tile([P, NC_D, DFF], BF, tag="W1")
        nc.scalar.dma_start(out=W1s, in_=W1d.rearrange("(c p) o -> p c o", p=P))
        W2s = wpool.tile([P, 16, D], BF, tag="W2")
        nc.scalar.dma_start(out=W2s, in_=W2d.rearrange("(c p) o -> p c o", p=P))

        # packed smalls: cols [0:12]=What' (12 q-tile cols: k*6+lt-1),
        # [12:16]=bq4, [16:20]=bk4, [20:36]=b1_16
        smalls = consts.tile([P, 36], F32, tag="smalls")
        nc.sync.dma_start(out=smalls, in_=smalls_d[:, :])
        What = smalls[:, 0:12]
        bq4 = smalls[:, 12:16]
        bk4 = smalls[:, 16:20]
        b1_16 = smalls[:, 20:36]
        masks = consts.tile([P, NQT, WIN], BF, tag="masks")
        nc.sync.dma_start(out=masks, in_=masks_d.rearrange("p (n w) -> p n w", n=NQT))
        # bias rows broadcast
        bin_b = consts.tile([P, D], F32, tag="bin_b")
        nc.sync.dma_start(out=bin_b, in_=_bcast_ap(rows_d[0, :]))
        bv_b = consts.tile([P, D], F32, tag="bv_b")
        nc.sync.dma_start(out=bv_b, in_=_bcast_ap(rows_d[1, :]))
        b2k_b = []
        for k in range(2):
            t = consts.tile([P, D], F32, tag=f"b2k{k}_b")
            nc.sync.dma_start(out=t, in_=_bcast_ap(rows_d[2 + k, :]))
            b2k_b.append(t)
        if add_bo:
            ones_r = consts.tile([1, P], BF, tag="ones_r")
            nc.vector.memset(ones_r, 1.0)
            bo_rf = consts.tile([1, D], F32, tag="bo_rf")
            nc.sync.dma_start(out=bo_rf, in_=rows_d[4:5, :])
            bo_rb = consts.tile([1, D], BF, tag="bo_rb")
            nc.vector.tensor_copy(out=bo_rb, in_=bo_rf)

        def ln_stats(in_ap, mv_ap):
            """bn_stats+aggr into mv_ap [128,2] = [mean, var]."""
            st = small.tile([P, 6], F32, tag="st6")
            nc.vector.bn_stats(out=st, in_=in_ap)
            nc.vector.bn_aggr(out=mv_ap, in_=st)

        def ln_rstd(in_ap):
            """mean[128,1], rstd[128,1] (per-tile; Sqrt + reciprocal)."""
            mv = small.tile([P, 2], F32, tag="mv")
            ln_stats(in_ap, mv)
            sd = small.tile([P, 1], F32, tag="sd")
            nc.scalar.activation(out=sd, in_=mv[:, 1:2], func=AF.Sqrt, bias=eps_t)
            rstd = small.tile([P, 1], F32, tag="rstd")
            nc.vector.reciprocal(out=rstd, in_=sd)
            return mv[:, 0:1], rstd

        def batch_rsqrt(mvb, n):
            """mvb [128, 2n] pairs -> rstd [128, n] in one ACT Sqrt + recip."""
            vcols = mvb.rearrange("p (t two) -> p two t", two=2)[:, 1, 0:n]
            sd = small.tile([P, 8], F32, tag="sdb")
            nc.scalar.activation(out=sd[:, 0:n], in_=vcols, func=AF.Sqrt, bias=eps_t)
            rst = small.tile([P, 8], F32, tag="rstb")
            nc.vector.reciprocal(out=rst[:, 0:n], in_=sd[:, 0:n])
            return rst

        # ---- A) X, LN_kv, Xk (both streams) ----
        lnkv = ln_nat_p.tile([P, NSH, D], BF, tag="ln_nat")
        Xk = acts.tile([P, 12, D], BF, tag="Xk")
        for mt in range(NSH):
            ps = psA.tile([P, D], F32, tag="psA")
            for c in range(NC_D):
                nc.tensor.matmul(
                    ps, lhsT=xmT_s[:, c, mt * P:(mt + 1) * P], rhs=Ws["Win"][:, c, :],
                    start=(c == 0), stop=(c == NC_D - 1))
            if add_bin:
                psb = sm2.tile([P, D], F32, tag="Xpsb")
                nc.vector.tensor_tensor(out=psb, in0=ps, in1=bin_b, op=OP.add)
            else:
                psb = ps
            mean, rstd = ln_rstd(psb)
            nc.vector.tensor_scalar(
                out=lnkv[:, mt, :], in0=psb, scalar1=mean, scalar2=rstd,
                op0=OP.subtract, op1=OP.mult)
            if 1 <= mt <= NQT:
                nc.scalar.activation(
                    out=Xk[:, mt - 1, :], in_=psb, func=AF.Copy,
                    scale=What[:, mt - 1:mt])
                nc.scalar.activation(
                    out=Xk[:, NQT + mt - 1, :], in_=psb, func=AF.Copy,
                    scale=What[:, NQT + mt - 1:NQT + mt])

        # ---- B) transpose LN_kv -> KVT [128, 4, TSH] ----
        def transpose_nat_to_T(src, dst, n, out_dt=BF):
            for mt in range(n):
                nc.sync.dma_start_transpose(
                    out=dst[:, :, mt * P:(mt + 1) * P], in_=src[:, mt, :])

        KVT = tT_p.tile([P, NC_D, TSH], BF, tag="tT")
        transpose_nat_to_T(lnkv, KVT, NSH)

        # ---- C) KT [128, 4, TSH] and V halo tiles ----
        KT = ktp.tile([P, NC_D, TSH], BF, tag="KT")
        for co in range(NC_D):
            for tch in range(2):
                ps = psA.tile([P, D], F32, tag="psA")
                for c in range(NC_D):
                    nc.tensor.matmul(
                        ps, lhsT=Ws["Wk"][:, c, co * P:(co + 1) * P],
                        rhs=KVT[:, c, tch * D:(tch + 1) * D],
                        start=(c == 0), stop=(c == NC_D - 1))
                nc.scalar.activation(
                    out=KT[:, co, tch * D:(tch + 1) * D], in_=ps,
                    func=AF.Identity, bias=bk4[:, co:co + 1])

        Vh = acts.tile([P, NV, D], BF, tag="Vh")
        for j in range(NV):
            s = 104 + j * P
            ps = psA.tile([P, D], F32, tag="psA")
            for c in range(NC_D):
                nc.tensor.matmul(
                    ps, lhsT=KVT[:, c, s:s + P], rhs=Ws["Wv"][:, c, :],
                    start=(c == 0), stop=(c == NC_D - 1))
            if add_bv:
                nc.vector.tensor_tensor(out=Vh[:, j, :], in0=ps, in1=bv_b, op=OP.add)
            else:
                nc.scalar.copy(out=Vh[:, j, :], in_=ps)

        inv_sqrt_dh = 1.0 / float(np.sqrt(DH))

        # ---- D') QT = Wq_eff^T @ KVT (+bq) -- LN(Xk) == LN(X), shared by
        # both streams (positive per-row gate cancels in LayerNorm)
        QT = ktp.tile([P, NC_D, TQ], BF, tag="QT")
        for co in range(NC_D):
            for tch, (t0w, w) in enumerate(((0, D), (D, TQ - D))):
                ps = psA.tile([P, D], F32, tag="psA")
                for c in range(NC_D):
                    nc.tensor.matmul(
                        ps[:, 0:w], lhsT=Ws["Wq"][:, c, co * P:(co + 1) * P],
                        rhs=KVT[:, c, P + t0w:P + t0w + w],
                        start=(c == 0), stop=(c == NC_D - 1))
                nc.scalar.activation(
                    out=QT[:, co, t0w:t0w + w], in_=ps[:, 0:w],
                    func=AF.Identity, bias=bq4[:, co:co + 1])

        # ---- E) attention (shared across streams) ----
        YT = acts.tile([P, NC_D, TQ], BF, tag="YT")
        for lt in range(1, NQT + 1):
            ws = lt * P - BAND
            q0 = (lt - 1) * P
            den = small.tile([P, H], F32, tag="den")
            pm_a = sm2.tile([P, H, WIN], BF, tag="pm_a")
            for h in range(H):
                hp, hc = 64 * (h % 2), h // 2
                ps = psB.tile([P, WIN], F32, tag="psB")
                nc.tensor.matmul(
                    ps, lhsT=QT[hp:hp + 64, hc, q0:q0 + P],
                    rhs=KT[hp:hp + 64, hc, ws:ws + WIN], start=True, stop=False)
                # accumulate additive band mask: ps += ident.T @ mask = mask
                nc.tensor.matmul(ps, lhsT=ident, rhs=masks[:, lt - 1, :],
                                 start=False, stop=True)
                nc.scalar.activation(out=pm_a[:, h, :], in_=ps, func=AF.Exp,
                                     scale=inv_sqrt_dh,
                                     accum_out=den[:, h:h + 1])
            r8 = small.tile([P, H], F32, tag="r8")
            nc.vector.reciprocal(out=r8, in_=den)
            psy = psD.tile([P, D], F32, tag="psD")
            for h in range(H):
                hp, hc = 64 * (h % 2), h // 2
                pms = sm2.tile([P, WIN], BF, tag="pms")
                nc.vector.tensor_scalar_mul(
                    out=pms, in0=pm_a[:, h, :], scalar1=r8[:, h:h + 1])
                ptp = psC.tile([P, 2 * P], BF, tag="psC")
                nc.tensor.transpose(ptp[:, 0:P], pms[:, 0:P], ident)
                nc.tensor.transpose(ptp[0:48, P:2 * P], pms[:, P:WIN], ident)
                pts = sm2.tile([P, 2 * P], BF, tag="pts")
                nc.vector.tensor_copy(out=pts, in_=ptp)
                nc.tensor.matmul(
                    psy[hp:hp + 64, hc * P:(hc + 1) * P],
                    lhsT=Vh[:, lt - 1, h * DH:(h + 1) * DH], rhs=pts[:, 0:P],
                    start=True, stop=False)
                nc.tensor.matmul(
                    psy[hp:hp + 64, hc * P:(hc + 1) * P],
                    lhsT=Vh[0:48, lt, h * DH:(h + 1) * DH],
                    rhs=pts[0:48, P:2 * P],
                    start=False, stop=True)
            nc.vector.tensor_copy(
                out=YT[:, :, q0:q0 + P],
                in_=psy.rearrange("p (c q) -> p c q", c=NC_D))

        # ---- F0) yo = YT.T @ Wo' (+bo') once, shared ----
        yo = acts.tile([P, NQT, D], BF, tag="yo")
        for mt in range(NQT):
            ps = psA.tile([P, D], F32, tag="psA")
            for c in range(NC_D):
                nc.tensor.matmul(
                    ps, lhsT=YT[:, c, mt * P:(mt + 1) * P], rhs=Ws["Wo"][:, c, :],
                    start=(c == 0), stop=(c == NC_D - 1 and not add_bo))
            if add_bo:
                nc.tensor.matmul(ps, lhsT=ones_r[:, 0:P], rhs=bo_rb,
                                 start=False, stop=True)
            nc.scalar.copy(out=yo[:, mt, :], in_=ps)

        # ---- F) y2_k = Xk + yo ; LN_f -> LNFT (both streams together) ----
        LNFT = tT_p.tile([P, NC_D, 2 * TQ], BF, tag="lnfT")
        y2 = stream_p.tile([P, 2 * NQT, D], BF, tag="y2")
        for k in range(2):
            lnf = ln_nat_p.tile([P, NQT, D], BF, tag="ln_nat")
            mvf = small.tile([P, 12], F32, tag="mvb")
            for grp in range(2):
                for mt in range(grp * 3, grp * 3 + 3):
                    eng = nc.vector if mt % 2 == 0 else nc.gpsimd
                    eng.tensor_tensor(
                        out=y2[:, k * NQT + mt, :], in0=yo[:, mt, :],
                        in1=Xk[:, k * NQT + mt, :], op=OP.add)
                    ln_stats(y2[:, k * NQT + mt, :], mvf[:, 2 * mt:2 * mt + 2])
                rstf = batch_rsqrt(mvf[:, grp * 6:grp * 6 + 6], 3)
                for mt in range(grp * 3, grp * 3 + 3):
                    j = mt - grp * 3
                    nc.vector.tensor_scalar(
                        out=lnf[:, mt, :], in0=y2[:, k * NQT + mt, :],
                        scalar1=mvf[:, 2 * mt:2 * mt + 1], scalar2=rstf[:, j:j + 1],
                        op0=OP.subtract, op1=OP.mult)
            for mt in range(NQT):
                nc.sync.dma_start_transpose(
                    out=LNFT[:, :, (k * NQT + mt) * P:(k * NQT + mt + 1) * P],
                    in_=lnf[:, mt, :])

        # ---- G) FFN over all 12 q-tiles (both streams) ----
        for tch in range(3):
            H1g = h1p.tile([P, 16, D], BF, tag="H1g")
            for dh in range(16):
                ps = psA.tile([P, D], F32, tag="psA")
                for c in range(NC_D):
                    nc.tensor.matmul(
                        ps, lhsT=W1s[:, c, dh * P:(dh + 1) * P],
                        rhs=LNFT[:, c, tch * D:(tch + 1) * D],
                        start=(c == 0), stop=(c == NC_D - 1))
                nc.scalar.activation(out=H1g[:, dh, :], in_=ps,
                                     func=AF.Gelu, scale=1.0 / SC,
                                     bias=b1_16[:, dh:dh + 1])
            for s0 in range(0, 4, 2):
                mvg = small.tile([P, 4], F32, tag="mvg")
                y3bs = []
                for j in range(2):
                    mtg = tch * 4 + s0 + j
                    ps = psA.tile([P, D], F32, tag="psA")
                    for dh in range(16):
                        nc.tensor.matmul(
                            ps, lhsT=H1g[:, dh, (s0 + j) * P:(s0 + j + 1) * P],
                            rhs=W2s[:, dh, :], start=(dh == 0), stop=(dh == 15))
                    y3 = outp.tile([P, D], F32, tag="y3")
                    nc.vector.tensor_tensor(out=y3, in0=ps, in1=y2[:, mtg, :],
                                            op=OP.add)
                    y3b = outp.tile([P, D], F32, tag="y3b")
                    nc.vector.tensor_tensor(out=y3b, in0=y3,
                                            in1=b2k_b[mtg // NQT], op=OP.add)
                    ln_stats(y3b, mvg[:, 2 * j:2 * j + 2])
                    y3bs.append(y3b)
                rstg = batch_rsqrt(mvg, 2)
                for j in range(2):
                    mtg = tch * 4 + s0 + j
                    o_sb = outp.tile([P, D], F32, tag="o_sb")
                    nc.vector.tensor_scalar(
                        out=o_sb, in0=y3bs[j], scalar1=mvg[:, 2 * j:2 * j + 1],
                        scalar2=rstg[:, j:j + 1],
                        op0=OP.subtract, op1=OP.mult)
                    nc.sync.dma_start(out=out_t[mtg], in_=o_sb)

    nc.finalize()
    return nc


_PROG_CACHE = {}


def kernel(**inputs) -> np.ndarray:
    f32 = np.float32
    bf = ml_dtypes.bfloat16
    fp8 = ml_dtypes.float8_e4m3
    x_m = np.asarray(inputs["x_m"], f32)
    A = np.asarray(inputs["A"], f32)
    g = {kk: np.asarray(v, f32) for kk, v in inputs.items()}

    # fold LN affine params into following matmuls (exact algebra)
    Wq = g["ln_q_g"][:, None] * g["Wq"]
    bq = g["bq"] + g["ln_q_b"] @ g["Wq"]
    Wk = g["ln_kv_g"][:, None] * g["Wk"]
    bk = g["bk"] + g["ln_kv_b"] @ g["Wk"]
    Wv = g["ln_kv_g"][:, None] * g["Wv"]
    bv = g["bv"] + g["ln_kv_b"] @ g["Wv"]
    W1 = g["ln_f_g"][:, None] * g["W1"]
    b1 = g["b1"] + g["ln_f_b"] @ g["W1"]

    add_bo = bool(np.any(g["bo"] != 0.0))
    add_bin = bool(np.any(g["b_in"] != 0.0))
    add_bv = bool(np.any(bv != 0.0))
    key = (add_bo, add_bin, add_bv)
    if key not in _PROG_CACHE:
        _PROG_CACHE[key] = build_program(add_bo, add_bin=add_bin, add_bv=add_bv)
    nc = _PROG_CACHE[key]

    common = {
        "Win": np.ascontiguousarray(g["W_in"].astype(bf)),
        "Wq": np.ascontiguousarray(Wq.astype(bf)),
        "Wk": np.ascontiguousarray(Wk.astype(bf)),
        "Wv": np.ascontiguousarray(Wv.astype(bf)),
        "Wo": np.ascontiguousarray((SC * g["Wo"]).astype(bf)),
        "W1": np.ascontiguousarray((SC * W1).astype(bf)),
        "W2": np.ascontiguousarray((SC * g["W2"]).astype(bf)),
    }

    # gate: What' = 128*sigmoid(6(A-0.5))  [B, T, K]
    What = SC / (1.0 + np.exp(-6.0 * (A - 0.5)))
    # padded transposed input [B, 512, T+256]
    xmp = np.zeros((B, D, T + 2 * P), f32)
    xmp[:, :, P:P + T] = np.transpose(x_m, (0, 2, 1))

    # band masks per (th, lt): [128 q-part, 176 key-window]
    jj = np.arange(WIN)
    pp = np.arange(P)
    band = ((jj[None, :] >= pp[:, None]) & (jj[None, :] <= pp[:, None] + 2 * BAND))

    in_maps = []
    for c in range(8):
        b, th = c // 2, c % 2
        im = dict(common)
        im["xmT"] = np.ascontiguousarray(
            xmp[b, :, th * TQ:th * TQ + TSH].astype(bf))
        sm = np.zeros((P, 36), f32)
        for k in range(2):
            for lt in range(NQT):
                sm[:, k * NQT + lt] = What[b, th * TQ + lt * P:th * TQ + (lt + 1) * P, k]
        sm[:, 12:16] = bq.reshape(4, P).T
        sm[:, 16:20] = bk.reshape(4, P).T
        sm[:, 20:36] = b1.reshape(16, P).T
        im["smalls"] = sm
        rows = np.stack([g["b_in"], bv,
                         SC * (g["b2"] + g["spk_tags"][0]),
                         SC * (g["b2"] + g["spk_tags"][1]),
                         SC * g["bo"]])
        im["rows"] = rows.astype(f32)
        mk = np.zeros((P, NQT, WIN), f32)
        for lt in range(1, NQT + 1):
            ws_true = th * TQ + lt * P - BAND - P  # true T coord of window col 0
            valid = (jj[None, :] + ws_true >= 0) & (jj[None, :] + ws_true < T)
            mk[:, lt - 1, :] = np.where(band & valid, 0.0, -1e30)
        im["masks"] = np.ascontiguousarray(mk.reshape(P, NQT * WIN).astype(bf))
        in_maps.append(im)

    res = run_bass_kernel_spmd(nc, in_maps, core_ids=list(range(8)))
    out = np.zeros((B, KSP * T, D), f32)
    gs, bs = g["ln_s_g"], g["ln_s_b"]
    for c in range(8):
        b, th = c // 2, c % 2
        r = res.results[c]["out"]
        for k in range(2):
            out[b, k * T + th * TQ:k * T + (th + 1) * TQ] = \
                r[k * TQ:(k + 1) * TQ] * gs + bs
    return out
